# revision 7
# baseline (speedup 1.0000x reference)
"""Fused Add + LayerNorm + Matmul block for Trainium2, 8 NeuronCores.

Reference computation (per problem nn_AddlnMatmulBlock_36558761623582):
    out_add = x1 + x2                      # [B=4, M=2048, N=1024]
    mean, rstd = layernorm stats over N    # [B, M]
    ln = (out_add - mean) * rstd * gamma + beta
    out = ln @ w + b                       # [B, M, D=4096]
    returns (out_add, mean, rstd, out)

Sharding: data-parallel over the 8192 rows (B*M), 1024 rows per core.
w/b replicated per core; gamma/beta folded into w/b on the host
(w_eff = gamma[:,None]*w, b_eff = b + beta@w), exact for any gamma/beta.

Per-core kernel: rows-on-partitions LayerNorm via bn_stats, PE-transpose
of x_hat into [N-on-partitions] tiles, bf16 matmul (1 cycle/row; fp32
and f32r run at 4x/2x cycles per row on TRN2) with fp32 PSUM
K-accumulation, bias add on PSUM drain.  out_add/mean/rstd stay fp32
exact; only the matmul operands are rounded to bf16 (~2e-3 rel).

Engine-stream discipline (sequencers execute in order, so a waiting
dma_start blocks everything behind it on that engine):
  - SP ring: all loads pre-issued in need-order (x0..x3, b, w0..w7,
    x4..x7 — HW queues are FIFO so completion follows this order),
    then the big out stores in drain order.
  - ACT ring: phase-1 only: per-tile sqrt + oadd/mean/rstd stores
    (all ready early, never behind a long-waiting store).
All compute overlap is dependency-driven by the Tile scheduler; the
phase split only shapes per-engine issue order.
"""

import numpy as np

import concourse.bacc as bacc
import concourse.bass as bass
import concourse.tile as tile
from concourse import mybir
from concourse.bass_utils import run_bass_kernel_spmd
from concourse.masks import make_identity

EPS = 1e-5
B, M, N, D = 4, 2048, 1024, 4096
N_CORES = 8
ROWS = B * M // N_CORES  # 1024 rows per core
MT = ROWS // 128  # 8 row tiles per core
KT = N // 128  # 8 contraction tiles
DT = D // 512  # 8 psum column tiles of 512

f32 = mybir.dt.float32
bf16 = mybir.dt.bfloat16


def _build():
    nc = bacc.Bacc(
        "TRN2", target_bir_lowering=False, debug=False, num_devices=N_CORES
    )

    x1_d = nc.dram_tensor("x1", [ROWS, N], f32, kind="ExternalInput").ap()
    x2_d = nc.dram_tensor("x2", [ROWS, N], f32, kind="ExternalInput").ap()
    w_d = nc.dram_tensor("w", [N, D], bf16, kind="ExternalInput").ap()
    b_d = nc.dram_tensor("b", [D], f32, kind="ExternalInput").ap()
    oadd_d = nc.dram_tensor("out_add", [ROWS, N], f32, kind="ExternalOutput").ap()
    mean_d = nc.dram_tensor("mean", [ROWS], f32, kind="ExternalOutput").ap()
    rstd_d = nc.dram_tensor("rstd", [ROWS], f32, kind="ExternalOutput").ap()
    out_d = nc.dram_tensor("out", [ROWS, D], f32, kind="ExternalOutput").ap()

    with tile.TileContext(nc) as tc:
        with (
            tc.tile_pool(name="singles", bufs=1) as singles,
            tc.tile_pool(name="x1p", bufs=8) as x1p,
            tc.tile_pool(name="x2p", bufs=8) as x2p,
            tc.tile_pool(name="stat", bufs=3) as stat,
            tc.tile_pool(name="xht", bufs=8) as xhtp,
            tc.tile_pool(name="outp", bufs=3) as outp,
            tc.tile_pool(name="ptr", bufs=2, space="PSUM") as ptr,
            tc.tile_pool(name="pmm", bufs=5, space="PSUM") as pmm,
        ):
            ident = singles.tile([128, 128], f32)
            make_identity(nc, ident)

            eps_t = singles.tile([128, 1], f32)
            nc.vector.memset(eps_t, EPS)

            # ---- phase 0: pre-issue every load on the SP ring ----
            xtiles = {}

            def issue_x_loads(i):
                rows = slice(i * 128, (i + 1) * 128)
                oadd = x1p.tile([128, N], f32, tag="oadd")
                xh = x2p.tile([128, N], f32, tag="xh")
                nc.sync.dma_start(out=oadd, in_=x1_d[rows, :])
                nc.scalar.dma_start(out=xh, in_=x2_d[rows, :])
                xtiles[i] = (oadd, xh)

            issue_x_loads(0)
            issue_x_loads(1)

            b_bc = singles.tile([128, D], f32)
            nc.sync.dma_start(
                out=b_bc,
                in_=bass.AP(
                    tensor=b_d.tensor, offset=b_d.offset, ap=[[0, 128]] + list(b_d.ap)
                ),
            )

            ring = [nc.sync, nc.scalar]
            w_r = []
            for k in range(KT):
                wk = singles.tile([128, D], bf16, tag=f"w{k}")
                ring[k % 2].dma_start(out=wk, in_=w_d[k * 128 : (k + 1) * 128, :])
                w_r.append(wk)
            for i in range(2, MT):
                issue_x_loads(i)

            # ---- phase 1: LN + transposes for every tile ----
            xhTs = []
            for i in range(MT):
                rows = slice(i * 128, (i + 1) * 128)
                oadd, xh = xtiles.pop(i)

                nc.gpsimd.tensor_add(out=oadd, in0=oadd, in1=xh)

                stats = stat.tile([128, 2, 6], f32)
                nc.vector.bn_stats(out=stats[:, 0, :], in_=oadd[:, 0:512])
                nc.vector.bn_stats(out=stats[:, 1, :], in_=oadd[:, 512:1024])
                mv = stat.tile([128, 2], f32)
                nc.vector.bn_aggr(out=mv, in_=stats)
                mean = mv[:, 0:1]
                var = mv[:, 1:2]

                rstd = stat.tile([128, 1], f32)
                nc.scalar.activation(
                    out=rstd,
                    in_=var,
                    func=mybir.ActivationFunctionType.Sqrt,
                    bias=eps_t,
                    scale=1.0,
                )
                nc.vector.reciprocal(out=rstd, in_=rstd)

                nc.scalar.dma_start(out=oadd_d[rows, :], in_=oadd)
                nc.scalar.dma_start(out=mean_d[rows], in_=mean)
                nc.scalar.dma_start(out=rstd_d[rows], in_=rstd)

                # x_hat = (oadd - mean) * rstd, in place over the x2 tile
                nc.vector.tensor_scalar(
                    out=xh,
                    in0=oadd,
                    scalar1=mean,
                    scalar2=rstd,
                    op0=mybir.AluOpType.subtract,
                    op1=mybir.AluOpType.mult,
                )

                xhT = xhtp.tile([128, KT, 128], bf16)
                for k in range(KT):
                    pt = ptr.tile([128, 128], f32)
                    nc.tensor.transpose(pt, xh[:, k * 128 : (k + 1) * 128], ident)
                    nc.vector.tensor_copy(out=xhT[:, k, :], in_=pt)
                xhTs.append(xhT)

            # ---- phase 2: matmul sweep, out stores on both rings ----
            for i in range(MT):
                rows = slice(i * 128, (i + 1) * 128)
                xhT = xhTs[i]
                for dd in range(DT // 4):  # quads of 512-col slices -> 2048 store
                    osb = outp.tile([128, 2048], f32)
                    for q in range(4):
                        d = dd * 4 + q
                        cols = slice(d * 512, (d + 1) * 512)
                        pm = pmm.tile([128, 512], f32)
                        for k in range(KT):
                            nc.tensor.matmul(
                                pm,
                                xhT[:, k, :],
                                w_r[k][:, cols],
                                start=(k == 0),
                                stop=(k == KT - 1),
                            )
                        nc.vector.tensor_add(
                            out=osb[:, q * 512 : (q + 1) * 512],
                            in0=pm,
                            in1=b_bc[:, cols],
                        )
                    ring[(i + dd) % 2].dma_start(
                        out=out_d[rows, dd * 2048 : (dd + 1) * 2048], in_=osb
                    )

    nc.compile()
    return nc


_NC = None


def make_in_maps(x1, x2, w, b, gamma, beta):
    import ml_dtypes

    x1 = np.ascontiguousarray(np.asarray(x1, dtype=np.float32).reshape(B * M, N))
    x2 = np.ascontiguousarray(np.asarray(x2, dtype=np.float32).reshape(B * M, N))
    w = np.asarray(w, dtype=np.float32)
    b = np.asarray(b, dtype=np.float32)
    gamma = np.asarray(gamma, dtype=np.float32)
    beta = np.asarray(beta, dtype=np.float32)

    # fold gamma/beta into w/b (exact when gamma=1, beta=0)
    w_eff = np.ascontiguousarray((gamma[:, None] * w).astype(ml_dtypes.bfloat16))
    b_eff = (b.astype(np.float64) + beta.astype(np.float64) @ w.astype(np.float64)).astype(
        np.float32
    )

    in_maps = []
    for c in range(N_CORES):
        rows = slice(c * ROWS, (c + 1) * ROWS)
        in_maps.append(
            {
                "x1": x1[rows],
                "x2": x2[rows],
                "w": w_eff,
                "b": b_eff,
            }
        )
    return in_maps


def kernel(x1, x2, w, b, gamma, beta):
    global _NC
    if _NC is None:
        _NC = _build()

    in_maps = make_in_maps(x1, x2, w, b, gamma, beta)
    res = run_bass_kernel_spmd(_NC, in_maps, list(range(N_CORES)))

    out_add = np.empty((B * M, N), dtype=np.float32)
    mean = np.empty((B * M,), dtype=np.float32)
    rstd = np.empty((B * M,), dtype=np.float32)
    out = np.empty((B * M, D), dtype=np.float32)
    for c in range(N_CORES):
        rows = slice(c * ROWS, (c + 1) * ROWS)
        r = res.results[c]
        out_add[rows] = r["out_add"]
        mean[rows] = r["mean"]
        rstd[rows] = r["rstd"]
        out[rows] = r["out"]

    return (
        out_add.reshape(B, M, N),
        mean.reshape(B, M),
        rstd.reshape(B, M),
        out.reshape(B, M, D),
    )


# revision 8
# speedup vs baseline: 1.1783x; 1.1783x over previous
"""Fused Add + LayerNorm + Matmul block for Trainium2, 8 NeuronCores.

Reference computation (per problem nn_AddlnMatmulBlock_36558761623582):
    out_add = x1 + x2                      # [B=4, M=2048, N=1024]
    mean, rstd = layernorm stats over N    # [B, M]
    ln = (out_add - mean) * rstd * gamma + beta
    out = ln @ w + b                       # [B, M, D=4096]
    returns (out_add, mean, rstd, out)

Sharding: data-parallel over the 8192 rows (B*M), 1024 rows per core.
w/b replicated per core; gamma/beta folded into w/b on the host
(w_eff = gamma[:,None]*w, b_eff = b + beta@w), exact for any gamma/beta.

Per-core kernel: rows-on-partitions LayerNorm via bn_stats, PE-transpose
of x_hat into [N-on-partitions] tiles, bf16 matmul (1 cycle/row; fp32
and f32r run at 4x/2x cycles per row on TRN2) with fp32 PSUM
K-accumulation, bias add on PSUM drain.  out_add/mean/rstd stay fp32
exact; only the matmul operands are rounded to bf16 (~2e-3 rel).

Engine-stream discipline (sequencers execute in order, so a waiting
dma_start blocks everything behind it on that engine):
  - SP ring: all loads pre-issued in need-order (x0..x3, b, w0..w7,
    x4..x7 — HW queues are FIFO so completion follows this order),
    then the big out stores in drain order.
  - ACT ring: phase-1 only: per-tile sqrt + oadd/mean/rstd stores
    (all ready early, never behind a long-waiting store).
All compute overlap is dependency-driven by the Tile scheduler; the
phase split only shapes per-engine issue order.
"""

import numpy as np

import concourse.bacc as bacc
import concourse.bass as bass
import concourse.tile as tile
from concourse import mybir
from concourse.bass_utils import run_bass_kernel_spmd
from concourse.masks import make_identity

EPS = 1e-5
B, M, N, D = 4, 2048, 1024, 4096
N_CORES = 8
ROWS = B * M // N_CORES  # 1024 rows per core
MT = ROWS // 128  # 8 row tiles per core
KT = N // 128  # 8 contraction tiles
DT = D // 512  # 8 psum column tiles of 512

f32 = mybir.dt.float32
bf16 = mybir.dt.bfloat16


def _build():
    nc = bacc.Bacc(
        "TRN2", target_bir_lowering=False, debug=False, num_devices=N_CORES
    )

    x1_d = nc.dram_tensor("x1", [ROWS, N], f32, kind="ExternalInput").ap()
    x2_d = nc.dram_tensor("x2", [ROWS, N], f32, kind="ExternalInput").ap()
    w_d = nc.dram_tensor("w", [N, D], bf16, kind="ExternalInput").ap()
    b_d = nc.dram_tensor("b", [D], f32, kind="ExternalInput").ap()
    oadd_d = nc.dram_tensor("out_add", [ROWS, N], f32, kind="ExternalOutput").ap()
    mean_d = nc.dram_tensor("mean", [ROWS], f32, kind="ExternalOutput").ap()
    rstd_d = nc.dram_tensor("rstd", [ROWS], f32, kind="ExternalOutput").ap()
    out_d = nc.dram_tensor("out", [ROWS, D], f32, kind="ExternalOutput").ap()

    with tile.TileContext(nc) as tc:
        with (
            tc.tile_pool(name="singles", bufs=1) as singles,
            tc.tile_pool(name="x1p", bufs=8) as x1p,
            tc.tile_pool(name="x2p", bufs=8) as x2p,
            tc.tile_pool(name="stat", bufs=3) as stat,
            tc.tile_pool(name="xht", bufs=8) as xhtp,
            tc.tile_pool(name="outp", bufs=3) as outp,
            tc.tile_pool(name="ptr", bufs=2, space="PSUM") as ptr,
            tc.tile_pool(name="pmm", bufs=5, space="PSUM") as pmm,
        ):
            ident = singles.tile([128, 128], f32)
            make_identity(nc, ident)

            eps_t = singles.tile([128, 1], f32)
            nc.vector.memset(eps_t, EPS)

            # ---- phase 0: pre-issue loads in need order ----

            ring = [nc.sync, nc.scalar]

            # SP-side x1 tiles, issued lazily in need order below
            x1tiles = {}

            def issue_x1(i):
                rows = slice(i * 128, (i + 1) * 128)
                oadd = x1p.tile([128, N], f32, tag="oadd")
                nc.sync.dma_start(out=oadd, in_=x1_d[rows, :])
                x1tiles[i] = oadd

            # ACT-side x2 tiles; keep ACT ring backlog small so the
            # per-tile sqrt is never stuck behind a full descriptor ring
            x2tiles = {}

            def issue_x2(i):
                rows = slice(i * 128, (i + 1) * 128)
                xh = x2p.tile([128, N], f32, tag="xh")
                nc.scalar.dma_start(out=xh, in_=x2_d[rows, :])
                x2tiles[i] = xh

            w_r = [None] * KT

            def issue_w(k, eng):
                wk = singles.tile([128, D], bf16, tag=f"w{k}")
                eng.dma_start(out=wk, in_=w_d[k * 128 : (k + 1) * 128, :])
                w_r[k] = wk

            b_bc = singles.tile([128, D], f32)

            # SP ring, ordered by first-need time
            issue_x1(0)
            issue_x1(1)
            issue_w(0, nc.sync)
            issue_w(2, nc.sync)
            nc.sync.dma_start(
                out=b_bc,
                in_=bass.AP(
                    tensor=b_d.tensor, offset=b_d.offset, ap=[[0, 128]] + list(b_d.ap)
                ),
            )
            issue_x1(2)
            issue_w(4, nc.sync)
            issue_x1(3)
            issue_w(5, nc.sync)
            issue_x1(4)
            issue_w(6, nc.sync)
            issue_x1(5)
            issue_w(7, nc.sync)
            issue_x1(6)
            issue_x1(7)

            # ACT ring: small pre-issue only
            issue_x2(0)
            issue_x2(1)
            issue_w(1, nc.scalar)
            issue_w(3, nc.scalar)

            # ---- phase 1: LN + transposes for every tile ----
            xhTs = []
            for i in range(MT):
                rows = slice(i * 128, (i + 1) * 128)
                if i + 2 < MT:
                    issue_x2(i + 2)
                oadd = x1tiles.pop(i)
                xh = x2tiles.pop(i)

                nc.gpsimd.tensor_add(out=oadd, in0=oadd, in1=xh)

                stats = stat.tile([128, 2, 6], f32)
                nc.vector.bn_stats(out=stats[:, 0, :], in_=oadd[:, 0:512])
                nc.vector.bn_stats(out=stats[:, 1, :], in_=oadd[:, 512:1024])
                mv = stat.tile([128, 2], f32)
                nc.vector.bn_aggr(out=mv, in_=stats)
                mean = mv[:, 0:1]
                var = mv[:, 1:2]

                rstd = stat.tile([128, 1], f32)
                nc.scalar.activation(
                    out=rstd,
                    in_=var,
                    func=mybir.ActivationFunctionType.Sqrt,
                    bias=eps_t,
                    scale=1.0,
                )
                nc.vector.reciprocal(out=rstd, in_=rstd)

                nc.scalar.dma_start(out=oadd_d[rows, :], in_=oadd)
                nc.scalar.dma_start(out=mean_d[rows], in_=mean)
                nc.scalar.dma_start(out=rstd_d[rows], in_=rstd)

                # x_hat = (oadd - mean) * rstd, in place over the x2 tile
                nc.vector.tensor_scalar(
                    out=xh,
                    in0=oadd,
                    scalar1=mean,
                    scalar2=rstd,
                    op0=mybir.AluOpType.subtract,
                    op1=mybir.AluOpType.mult,
                )

                xhT = xhtp.tile([128, KT, 128], bf16)
                for k in range(KT):
                    pt = ptr.tile([128, 128], f32)
                    nc.tensor.transpose(pt, xh[:, k * 128 : (k + 1) * 128], ident)
                    nc.vector.tensor_copy(out=xhT[:, k, :], in_=pt)
                xhTs.append(xhT)

            # ---- phase 2: matmul sweep, out stores on both rings ----
            for i in range(MT):
                rows = slice(i * 128, (i + 1) * 128)
                xhT = xhTs[i]
                for dd in range(DT // 4):  # quads of 512-col slices -> 2048 store
                    osb = outp.tile([128, 2048], f32)
                    for q in range(4):
                        d = dd * 4 + q
                        cols = slice(d * 512, (d + 1) * 512)
                        pm = pmm.tile([128, 512], f32)
                        for k in range(KT):
                            nc.tensor.matmul(
                                pm,
                                xhT[:, k, :],
                                w_r[k][:, cols],
                                start=(k == 0),
                                stop=(k == KT - 1),
                            )
                        nc.vector.tensor_add(
                            out=osb[:, q * 512 : (q + 1) * 512],
                            in0=pm,
                            in1=b_bc[:, cols],
                        )
                    ring[(i + dd) % 2].dma_start(
                        out=out_d[rows, dd * 2048 : (dd + 1) * 2048], in_=osb
                    )

    nc.compile()
    return nc


_NC = None


def make_in_maps(x1, x2, w, b, gamma, beta):
    import ml_dtypes

    x1 = np.ascontiguousarray(np.asarray(x1, dtype=np.float32).reshape(B * M, N))
    x2 = np.ascontiguousarray(np.asarray(x2, dtype=np.float32).reshape(B * M, N))
    w = np.asarray(w, dtype=np.float32)
    b = np.asarray(b, dtype=np.float32)
    gamma = np.asarray(gamma, dtype=np.float32)
    beta = np.asarray(beta, dtype=np.float32)

    # fold gamma/beta into w/b (exact when gamma=1, beta=0)
    w_eff = np.ascontiguousarray((gamma[:, None] * w).astype(ml_dtypes.bfloat16))
    b_eff = (b.astype(np.float64) + beta.astype(np.float64) @ w.astype(np.float64)).astype(
        np.float32
    )

    in_maps = []
    for c in range(N_CORES):
        rows = slice(c * ROWS, (c + 1) * ROWS)
        in_maps.append(
            {
                "x1": x1[rows],
                "x2": x2[rows],
                "w": w_eff,
                "b": b_eff,
            }
        )
    return in_maps


def kernel(x1, x2, w, b, gamma, beta):
    global _NC
    if _NC is None:
        _NC = _build()

    in_maps = make_in_maps(x1, x2, w, b, gamma, beta)
    res = run_bass_kernel_spmd(_NC, in_maps, list(range(N_CORES)))

    out_add = np.empty((B * M, N), dtype=np.float32)
    mean = np.empty((B * M,), dtype=np.float32)
    rstd = np.empty((B * M,), dtype=np.float32)
    out = np.empty((B * M, D), dtype=np.float32)
    for c in range(N_CORES):
        rows = slice(c * ROWS, (c + 1) * ROWS)
        r = res.results[c]
        out_add[rows] = r["out_add"]
        mean[rows] = r["mean"]
        rstd[rows] = r["rstd"]
        out[rows] = r["out"]

    return (
        out_add.reshape(B, M, N),
        mean.reshape(B, M),
        rstd.reshape(B, M),
        out.reshape(B, M, D),
    )
